# revision 7
# baseline (speedup 1.0000x reference)
"""DotGAT layer (segment-softmax GNN message passing) on 8 Trainium2 cores, v3.

Strategy (graph/data parallel per the sharding hint):
  - Nodes split into 8 contiguous ranges of 6272 (49 blocks of 128); each core
    owns the edges whose dst falls in its range.  Halo exchange is host-side
    data layout: each core receives its edges' source features as two streams,
    feature-major fp16 (logit path) and edge-major fp16 (value path).
  - v3 key idea: each 128-node block is split into four aligned 32-node
    SEGMENTS.  Every 128-edge chunk only contains edges of ONE segment, so all
    of its destinations fall in a fixed, SPMD-shared 32-column window.  The
    logit matmul streams only those 32 qk columns, the exp processes a packed
    [128, 4x32] group tile, the mask is applied POST-exp as a cheap 32-wide
    multiply (no identity matmul / +-30 PSUM trick needed), and the
    aggregation matmuls write 32-column / 32-partition slices.
  - Logits use the fused form e = z_src . (B z_dst) with B = tau Wk Wq^T, so
    q/k are never materialised.
  - Per chunk PE work: MM1 (logits, 32-col stream), MM3 (U += zet^T P, 32-col
    stream), MM4 (denominator, 1-col).  The mask matmul of v2 is gone.

The program is recompiled per call with all data-dependent sizes baked in
(SPMD: one instruction stream, 8 cores).
"""

import sys

sys.path.insert(0, "/opt/trn_rl_repo")

import numpy as np

N_NODES = 50000
N_EDGES_EXPECT = 800000
DIM = 128
N_CORES = 8
BLK = 128
SEGW = 32  # nodes per segment (aligned dst window)
NSEG = BLK // SEGW  # 4 segments per block
BLOCKS_PER_CORE = 49
NODES_PER_CORE = BLOCKS_PER_CORE * BLK  # 6272
N_PAD = NODES_PER_CORE * N_CORES  # 50176
TAU = 1.0 / np.sqrt(DIM)


def _prepare(z, Wq, bq, Wk, bk, Wv, bv, src, dst):
    """Host-side sharding: per-core edge grouping into (block, segment) chunks
    and the two source-feature streams (pure data movement, no arithmetic)."""
    z = np.asarray(z, np.float32)
    src = np.asarray(src, np.int64)
    dst = np.asarray(dst, np.int64)
    assert not np.any(np.asarray(bq)) and not np.any(np.asarray(bk)) and not np.any(
        np.asarray(bv)
    ), "v3 kernel assumes zero biases"

    zT16 = np.zeros((DIM, N_PAD + 1), np.float16)
    zT16[:, :N_NODES] = z.T.astype(np.float16)
    z16 = np.zeros((N_PAD + 1, DIM), np.float16)
    z16[:N_NODES] = z.astype(np.float16)

    NBS = BLOCKS_PER_CORE * NSEG  # block-segment slots per core (196)
    # per-core sorted edge lists + per (block,seg) counts
    per_core = []
    cnts = np.zeros((N_CORES, NBS), np.int64)
    for c in range(N_CORES):
        n0 = c * NODES_PER_CORE
        sel = (dst >= n0) & (dst < n0 + NODES_PER_CORE)
        es = src[sel]
        ed = dst[sel] - n0
        bs = ed >> 5  # combined block*4+seg index (0..195), since SEGW=32
        order = np.argsort(ed, kind="stable")
        es, ed, bs = es[order], ed[order], bs[order]
        np.add.at(cnts[c], bs, 1)
        per_core.append((es, ed))

    # shared schedule: chunks of <=128 edges per (block,seg), padded to the
    # max need over cores, at least 1 chunk per slot
    Cseg = np.maximum((-(-cnts // BLK)).max(axis=0), 1)  # [196]
    S = int(Cseg.sum())
    offs = np.concatenate([[0], np.cumsum(Cseg)]).astype(int)  # per-slot chunk offset

    WqT = (np.asarray(Wq, np.float32).T * TAU).astype(np.float16).copy()
    WkT = np.ascontiguousarray(np.asarray(Wk, np.float32).T).astype(np.float16)
    Wv16 = np.asarray(Wv, np.float32).astype(np.float16)
    iota32 = np.broadcast_to(
        np.arange(SEGW, dtype=np.float16), (BLK, SEGW)
    ).copy()  # [e, n] value = n (within segment)

    in_maps = []
    for c in range(N_CORES):
        es, ed = per_core[c]
        col = np.full(S * BLK, N_PAD, np.int64)  # pad -> zero feature row/col
        adj = np.full(S * BLK, -1.0, np.float32)  # pad -> matches no node
        cstart = np.concatenate([[0], np.cumsum(cnts[c])]).astype(int)
        for t in range(NBS):
            n = int(cnts[c][t])
            p0 = int(cstart[t])
            base = int(offs[t]) * BLK
            col[base : base + n] = es[p0 : p0 + n]
            adj[base : base + n] = (ed[p0 : p0 + n] - (t << 5)).astype(np.float32)
        ze = np.ascontiguousarray(zT16[:, col])  # [128, S*128] fp16
        zet = np.ascontiguousarray(
            z16[col].reshape(S, BLK, DIM).transpose(1, 0, 2).reshape(BLK, S * DIM)
        )  # [128(e), S*128] fp16
        dstadj = np.ascontiguousarray(
            adj.reshape(S, BLK).T.astype(np.float32)
        )  # [128(e), S]
        zq = np.ascontiguousarray(zT16[:, c * NODES_PER_CORE : (c + 1) * NODES_PER_CORE])
        in_maps.append(
            dict(ze=ze, zet=zet, dstadj=dstadj, zq=zq, WqT=WqT, WkT=WkT, Wv=Wv16,
                 iota=iota32)
        )
    consts = dict(Cseg=[int(x) for x in Cseg], S=S)
    return in_maps, consts


def _build(consts):
    import concourse.bacc as bacc
    import concourse.mybir as mybir
    import concourse.tile as tile

    dt = mybir.dt
    Alu = mybir.AluOpType
    Act = mybir.ActivationFunctionType

    Cseg = consts["Cseg"]
    S = consts["S"]
    NBS = BLOCKS_PER_CORE * NSEG
    offs = np.concatenate([[0], np.cumsum(Cseg)]).astype(int)
    # per-block chunk counts / offsets
    cblk = [int(sum(Cseg[b * NSEG : (b + 1) * NSEG])) for b in range(BLOCKS_PER_CORE)]
    boffs = np.concatenate([[0], np.cumsum(cblk)]).astype(int)

    # flat schedule: (s, b, seg, k, Cc)
    flat = []
    for t in range(NBS):
        b, seg = t >> 2, t & 3
        for k in range(Cseg[t]):
            flat.append((int(offs[t]) + k, b, seg, k, Cseg[t]))
    assert len(flat) == S

    nc = bacc.Bacc("TRN2", target_bir_lowering=False, debug=False, num_devices=N_CORES)

    ze = nc.declare_dram_parameter("ze", [128, S * BLK], dt.float16, isOutput=False)
    zet = nc.declare_dram_parameter("zet", [128, S * BLK], dt.float16, isOutput=False)
    dstadj = nc.declare_dram_parameter("dstadj", [128, S], dt.float32, isOutput=False)
    zq = nc.declare_dram_parameter("zq", [128, NODES_PER_CORE], dt.float16, isOutput=False)
    WqT = nc.declare_dram_parameter("WqT", [128, 128], dt.float16, isOutput=False)
    WkT = nc.declare_dram_parameter("WkT", [128, 128], dt.float16, isOutput=False)
    Wv = nc.declare_dram_parameter("Wv", [128, 128], dt.float16, isOutput=False)
    iota = nc.declare_dram_parameter("iota", [128, SEGW], dt.float16, isOutput=False)
    h = nc.declare_dram_parameter("h", [NODES_PER_CORE, DIM], dt.float16, isOutput=True)

    with tile.TileContext(nc) as tc:
        with tc.tile_pool(name="const", bufs=1) as constp:
            wqt_sb = constp.tile([128, 128], dt.float16)
            wkt_sb = constp.tile([128, 128], dt.float16)
            wv_sb = constp.tile([128, 128], dt.float16)
            iota_sb = constp.tile([128, SEGW], dt.float16)
            adj_sb = constp.tile([128, S], dt.float32)
            ones_sb = constp.tile([128, 1], dt.float16)
            one1_sb = constp.tile([1, 1], dt.float32)
            nc.vector.memset(one1_sb[:], 1.0)
            nc.sync.dma_start(wqt_sb[:], WqT[:])
            nc.sync.dma_start(wkt_sb[:], WkT[:])
            nc.sync.dma_start(wv_sb[:], Wv[:])
            nc.sync.dma_start(iota_sb[:], iota[:])
            nc.sync.dma_start(adj_sb[:], dstadj[:])
            nc.vector.memset(ones_sb[:], 1.0)

            # x[j, i] = (tau Wq Wk^T)[j, i]; per block qk = x^T z_own
            x_sb = constp.tile([128, 128], dt.float16)

            # ---- PE warm-up: ~6us of dense matmuls so the HAM clock gate
            # lifts the PE to 2.4 GHz before the main loop ----
            with tc.tile_pool(name="warm", bufs=4, space="PSUM") as wpool:
                for i in range(80):
                    wps = wpool.tile([128, 128], dt.float32, tag="w")
                    nc.tensor.matmul(
                        wps[:], lhsT=wqt_sb[:], rhs=wkt_sb[:], start=True, stop=True
                    )
                xp = wpool.tile([128, 128], dt.float32, tag="w")
                nc.tensor.matmul(xp[:], lhsT=wqt_sb[:], rhs=wkt_sb[:], start=True, stop=True)
                nc.scalar.copy(x_sb[:], xp[:])

            G = 4  # blocks per DMA group
            Gmax = max(
                sum(cblk[b0 : b0 + G]) for b0 in range(0, BLOCKS_PER_CORE, G)
            )
            zq_all = constp.tile([128, NODES_PER_CORE], dt.float16)
            nc.sync.dma_start(zq_all[:], zq[:])
            with (
                tc.tile_pool(name="zep", bufs=2) as zep,
                tc.tile_pool(name="zetp", bufs=2) as zetp,
                tc.tile_pool(name="qkp", bufs=3) as qkp,
                tc.tile_pool(name="mp", bufs=3) as mp,
                tc.tile_pool(name="pep", bufs=3) as pep,
                tc.tile_pool(name="ptp", bufs=3) as ptp,
                tc.tile_pool(name="usb", bufs=2) as usbp,
                tc.tile_pool(name="recp", bufs=2) as recp,
                tc.tile_pool(name="hp", bufs=2) as hp,
                tc.tile_pool(name="stps", bufs=3, space="PSUM") as stps,
                tc.tile_pool(name="ups", bufs=2, space="PSUM") as ups,
                tc.tile_pool(name="bkps", bufs=2, space="PSUM") as bkps,
            ):
                blk_state = {}

                def open_block(b):
                    bk = bkps.tile([128, 512], dt.float32, tag="bk")
                    nc.tensor.matmul(
                        bk[:, 0:128], lhsT=x_sb[:],
                        rhs=zq_all[:, b * 128 : (b + 1) * 128], start=True, stop=True
                    )
                    qk_sb = qkp.tile([128, 128], dt.float16, tag="qksb")
                    nc.scalar.copy(qk_sb[:], bk[:, 0:128])
                    u_ps = ups.tile([128, 128], dt.float32, tag="u")
                    blk_state[b] = dict(qk=qk_sb, u=u_ps, bk=bk)

                def close_block(b):
                    st = blk_state.pop(b)
                    u_sb = usbp.tile([128, 128], dt.float16, tag="usb")
                    nc.vector.tensor_copy(u_sb[:], st["u"][:])
                    bk = st["bk"]
                    nc.tensor.matmul(
                        bk[:, 128:256], lhsT=u_sb[:], rhs=wv_sb[:], start=True, stop=True
                    )
                    # transpose the denominator row [1,128] -> column [128,1]
                    drow_sb = usbp.tile([1, 128], dt.float32, tag="drow")
                    nc.vector.tensor_copy(drow_sb[:], bk[0:1, 256:384])
                    nc.tensor.matmul(
                        bk[:, 384:385], lhsT=drow_sb[:], rhs=one1_sb[:],
                        start=True, stop=True
                    )
                    rec = recp.tile([128, 2], dt.float32, tag="rec")
                    nc.vector.tensor_scalar(
                        out=rec[:, 0:1], in0=bk[:, 384:385], scalar1=1e-20, scalar2=None,
                        op0=Alu.add,
                    )
                    nc.vector.reciprocal(rec[:, 1:2], rec[:, 0:1])
                    ht = hp.tile([128, 128], dt.float16, tag="h")
                    nc.vector.tensor_scalar(
                        out=ht[:], in0=bk[:, 128:256], scalar1=rec[:, 1:2], scalar2=None,
                        op0=Alu.mult,
                    )
                    nc.sync.dma_start(h[b * 128 : (b + 1) * 128, :], ht[:])

                # iterate chunks in EXP-groups of 4 within each DMA group
                for b0 in range(0, BLOCKS_PER_CORE, G):
                    bl = list(range(b0, min(b0 + G, BLOCKS_PER_CORE)))
                    s0 = int(boffs[bl[0]])
                    nchunk = sum(cblk[b] for b in bl)
                    zeg = zep.tile([128, Gmax * 128], dt.float16, tag="ze")
                    nc.sync.dma_start(
                        zeg[:, : nchunk * 128], ze[:, s0 * 128 : (s0 + nchunk) * 128]
                    )
                    ztg = zetp.tile([128, Gmax * 128], dt.float16, tag="zet")
                    nc.sync.dma_start(
                        ztg[:, : nchunk * 128], zet[:, s0 * 128 : (s0 + nchunk) * 128]
                    )
                    chunks = flat[s0 : s0 + nchunk]
                    for g0 in range(0, nchunk, 4):
                        grp = chunks[g0 : g0 + 4]
                        ng = len(grp)
                        w = ng * SEGW
                        stt = stps.tile([128, NSEG * SEGW], dt.float32, tag="st")
                        pexp = pep.tile([128, NSEG * SEGW], dt.float16, tag="pe")
                        m4 = mp.tile([128, NSEG * SEGW], dt.float16, tag="m")
                        pt = ptp.tile([128, NSEG * SEGW], dt.float16, tag="pt")
                        # MM1 per chunk: narrow logits
                        for j, (s, b, seg, k, Cc) in enumerate(grp):
                            if b not in blk_state:
                                open_block(b)
                            loc = s - s0
                            nc.tensor.matmul(
                                stt[:, j * SEGW : (j + 1) * SEGW],
                                lhsT=zeg[:, loc * 128 : (loc + 1) * 128],
                                rhs=blk_state[b]["qk"][:, seg * SEGW : (seg + 1) * SEGW],
                                start=True, stop=True,
                            )
                            # mask: one-hot of dst within segment (0/1)
                            nc.vector.tensor_scalar(
                                out=m4[:, j * SEGW : (j + 1) * SEGW], in0=iota_sb[:],
                                scalar1=adj_sb[:, s : s + 1], scalar2=None,
                                op0=Alu.is_equal,
                            )
                        nc.scalar.activation(pexp[:, 0:w], stt[:, 0:w], Act.Exp)
                        nc.vector.tensor_mul(pt[:, 0:w], pexp[:, 0:w], m4[:, 0:w])
                        for j, (s, b, seg, k, Cc) in enumerate(grp):
                            st = blk_state[b]
                            loc = s - s0
                            nc.tensor.matmul(
                                st["u"][:, seg * SEGW : (seg + 1) * SEGW],
                                lhsT=ztg[:, loc * 128 : (loc + 1) * 128],
                                rhs=pt[:, j * SEGW : (j + 1) * SEGW],
                                start=(k == 0), stop=(k == Cc - 1),
                                skip_group_check=True,
                            )
                            nc.tensor.matmul(
                                st["bk"][0:1, 256 + seg * SEGW : 256 + (seg + 1) * SEGW],
                                lhsT=ones_sb[:],
                                rhs=pt[:, j * SEGW : (j + 1) * SEGW],
                                start=(k == 0), stop=(k == Cc - 1),
                                skip_group_check=True,
                            )
                            if seg == NSEG - 1 and k == Cc - 1:
                                close_block(b)

    nc.compile()
    return nc


def _install_ntff_hook():
    """The agent image's antenv lacks axon_hooks; recreate it and register
    the ctypes NTFF profile hook the boot would have installed."""
    import types

    if "antenv.axon_hooks" not in sys.modules:
        import antenv

        m = types.ModuleType("antenv.axon_hooks")
        m._hook = None
        m.set_axon_ntff_profile_hook = lambda h, _m=m: setattr(_m, "_hook", h)
        m.get_axon_ntff_profile_hook = lambda _m=m: _m._hook
        sys.modules["antenv.axon_hooks"] = m
        antenv.axon_hooks = m
    from antenv import axon_hooks

    if axon_hooks.get_axon_ntff_profile_hook() is None:
        from trn_agent_boot.trn_boot import _ntff_profile_via_ctypes

        hook = _ntff_profile_via_ctypes("/opt/axon/libaxon_pjrt.so")
        if hook is not None:
            axon_hooks.set_axon_ntff_profile_hook(hook)


def run(inputs, trace=False):
    """Returns (h [50000,128] float32, exec_time_ns or None)."""
    from concourse.bass_utils import run_bass_kernel_spmd

    if trace:
        try:
            _install_ntff_hook()
        except Exception as e:  # profiling is best-effort
            print(f"ntff hook install failed: {e}", file=sys.stderr)

    in_maps, consts = _prepare(**inputs)
    nc = _build(consts)
    res = run_bass_kernel_spmd(
        nc,
        [dict(m) for m in in_maps],
        list(range(N_CORES)),
        trace=trace,
    )
    hh = np.concatenate([r["h"] for r in res.results], axis=0)[:N_NODES]
    return np.ascontiguousarray(hh.astype(np.float32)), res.exec_time_ns


def kernel(**inputs) -> np.ndarray:
    hh, _ = run(inputs, trace=False)
    return hh
